# revision 47
# baseline (speedup 1.0000x reference)
"""Trainium2 Bass kernel for masked multi-head attention (b=2, n=2048, dim=1024, 16 heads).

Sharding: 8 cores = batch(2) x head-groups(4). Core c handles batch c//4 and
heads [4*(c%4), 4*(c%4)+4) as two head-PAIRS. Host sums the 4 partials/batch.

Key structure:
  - Host sorts positions by mask (kept first) and ZEROES x rows at masked
    positions. So q=k=v=0 for masked positions automatically: no mask-bias row
    and no mask-broadcast multiply on device. A masked query gets all-zero
    logits -> exp=1 uniform, matching the reference's uniform softmax; masked
    keys are excluded from numerator and denominator because their va row and
    kmask entry are 0. A host-side correction (vsc/me) adds the masked keys'
    v-sum and count for masked queries only.
  - One x buffer serves Q (nq cols), K and V (nk cols) - no duplicate load.
  - Scores S^T = ka^T @ qa with K=64 contraction; the two heads of a pair run
    CONCURRENTLY in the PE array via row tiling (partitions 0-63 / 64-127).
  - va per head is [key,128]: cols 0-63 = kmask replicated, cols 64-127 = v.
    So po = va^T @ E gives rows 0-63 = the softmax denominator replicated
    across 64 partitions and rows 64-127 = O^T: normalization is just
    reciprocal_approx_fast + one multiply on DVE (no broadcast matmul).
    All DVE source windows start at partition 0 except the final multiply's
    in0 (po rows 64-127); custom-DVE ops require base-partition-0 operands.
  - Scalar engine does ONLY exp; all PSUM evacuations run on Vector. The
    q/k projections run t-major across 6 held PSUM slots so they pace with
    the xs DMA chunks; pair-1 projections, the V projection, PV and the
    output projection are scheduled between score bursts so the PE stays
    saturated while the exp stream runs.
  - Output projection per q-chunk is interleaved mid-kernel (only the last
    chunk's PV+C sit past the final exp); y is written bf16 (host sums
    partials in fp32 and fills all fully-masked rows with a host-computed
    uniform-attention row).
"""

import numpy as np

import concourse.bacc as bacc
import concourse.bass as bass
import concourse.tile as tile
from concourse import mybir
from concourse.bass_utils import run_bass_kernel_spmd

F32 = mybir.dt.float32
BF16 = mybir.dt.bfloat16
EXP = mybir.ActivationFunctionType.Exp

N_CORES = 8
HEADS = 16
DH = 64
SCALE = DH ** -0.5


def build_nc(n=2048, d_model=1024, nq=None, nk=None, debug=False):
    """Build + compile the single-core Bass program (SPMD across 8 cores)."""
    dh = DH
    DT = d_model // 128          # contraction tiles for the projections
    if nq is None:
        nq = n
    if nk is None:
        nk = n
    NTK = nk // 128              # kept key tiles
    NCHQ = 3 if nq % 3 == 0 and nq // 3 <= 512 else (nq + 511) // 512
    qck = nq // NCHQ             # kept-query chunk size
    assert qck * NCHQ == nq and qck <= 512 and qck % 128 == 0
    if nk % 384 == 0:
        kck = 384
    elif nk % 512 == 0:
        kck = 512
    else:
        kck = 128
    NCHK = nk // kck
    nx = max(nq, nk)
    DCH = d_model // 512         # output-projection chunks
    QTPC = qck // 128            # q-tiles per chunk

    nc = bacc.Bacc("TRN2", target_bir_lowering=False, debug=False,
                   enable_asserts=False, num_devices=N_CORES)

    xs_d = nc.dram_tensor("xs", [128, DT * nx], BF16, kind="ExternalInput").ap()
    wc_d = nc.dram_tensor("wcat", [128, DT * 768 + 2 * d_model], BF16,
                          kind="ExternalInput").ap()  # [wqk | wv | wo]
    km_d = nc.dram_tensor("km64", [128, NTK * 64], BF16, kind="ExternalInput").ap()
    y_d = nc.dram_tensor("y", [nq, d_model], BF16, kind="ExternalOutput").ap()
    if debug:
        dbg = {nm: nc.dram_tensor(f"dbg_{nm}", shp, BF16,
                                  kind="ExternalOutput").ap()
               for nm, shp in [("qa0", [128, nq]), ("ka0", [128, nk]),
                               ("va0", [128, (nk // 128) * 2 * 128]),
                               ("otp0", [128, nq]), ("otp1", [128, nq]),
                               ("e00", [128, 2 * 512]), ("poA", [128, 512]),
                               ("poB", [128, 512]), ("rbA", [64, 512])]}

    with tile.TileContext(nc) as tc:
        with tc.tile_pool(name="persist", bufs=1) as P, \
             tc.tile_pool(name="pss", bufs=2, space="PSUM") as pss, \
             tc.tile_pool(name="pso", bufs=3, space="PSUM") as pso, \
             tc.tile_pool(name="eb", bufs=2 * NTK + 4) as eb, \
             tc.tile_pool(name="nrm", bufs=2) as nrm, \
             tc.tile_pool(name="ytp", bufs=3) as ytp:
            xs_sb = P.tile([128, DT, nx], BF16, tag="xs_sb")
            w_sb = P.tile([128, DT, 2, 256], BF16, tag="w_sb")
            wv_sb = P.tile([128, DT, 256], BF16, tag="wv_sb")
            wo_sb = P.tile([128, 2, d_model], BF16, tag="wo_sb")
            qa = [P.tile([128, nq], BF16, name=f"qa{g}", tag=f"qa{g}")
                  for g in range(2)]
            ka = [P.tile([128, nk], BF16, name=f"ka{g}", tag=f"ka{g}")
                  for g in range(2)]
            va = [P.tile([128, NTK, 2, 128], BF16, name=f"va{g}", tag=f"va{g}")
                  for g in range(2)]
            otp = [P.tile([128, nq], BF16, name=f"otp{g}", tag=f"otp{g}")
                   for g in range(2)]
            km_sb = P.tile([128, NTK, 64], BF16, tag="km_sb")

            # ---------------- input DMAs ----------------
            # Ordered so chunk t (wqk + both xs halves) lands ~in t order:
            # gpsimd carries wqk per-t, sync/scalar each carry half of xs t
            # (quarters for the first two chunks so the PE starts sooner).
            # wv / km follow; wo (stage C only) is deferred to mid-kernel.
            hx = nx // 2
            qx = hx // 2
            for t in range(DT):
                nc.gpsimd.dma_start(
                    out=w_sb[:, t, :, :].rearrange("p a b -> p (a b)"),
                    in_=wc_d[:, t * 512:(t + 1) * 512])
                if t < 2:
                    for qi in range(2):
                        nc.sync.dma_start(
                            out=xs_sb[:, t, qi * qx:(qi + 1) * qx],
                            in_=xs_d[:, t * nx + qi * qx:t * nx + (qi + 1) * qx])
                        nc.scalar.dma_start(
                            out=xs_sb[:, t, hx + qi * qx:hx + (qi + 1) * qx],
                            in_=xs_d[:, t * nx + hx + qi * qx:
                                     t * nx + hx + (qi + 1) * qx])
                else:
                    nc.sync.dma_start(out=xs_sb[:, t, 0:hx],
                                      in_=xs_d[:, t * nx:t * nx + hx])
                    nc.scalar.dma_start(out=xs_sb[:, t, hx:nx],
                                        in_=xs_d[:, t * nx + hx:(t + 1) * nx])
            nc.sync.dma_start(
                out=wv_sb.rearrange("p a b -> p (a b)"),
                in_=wc_d[:, DT * 512:DT * 768])
            nc.scalar.dma_start(out=km_sb.rearrange("p a b -> p (a b)"),
                                in_=km_d)

            wq = lambda t, g: w_sb[:, t, 0, g * 128:(g + 1) * 128]
            wk = lambda t, g: w_sb[:, t, 1, g * 128:(g + 1) * 128]
            wv = lambda t: wv_sb[:, t, :]

            # ---------------- emit helpers ----------------
            def emit_qk(g, which, j, evac):
                """One accumulation group: 128-row block (pair g) of q or k
                over chunk j, contraction over all DT x tiles; then evac."""
                w = wq if which == 0 else wk
                dst = qa[g] if which == 0 else ka[g]
                ck = qck if which == 0 else kck
                ps = pso.tile([128, ck], F32, name="psqk",
                              padded_shape=[128, 512], tag="pso")
                for t in range(DT):
                    nc.tensor.matmul(ps, lhsT=w(t, g),
                                     rhs=xs_sb[:, t, j * ck:(j + 1) * ck],
                                     start=(t == 0), stop=(t == DT - 1))
                evac(out=dst[:, j * ck:(j + 1) * ck], in_=ps)

            def v_thunks():
                """V projection for BOTH pairs: psv groups of 2 key tiles
                [128, 2, 256] (1 bank); slot-major accumulation; DVE evac.
                Plus the kmask fills of va cols 64:128 (gpsimd)."""
                th = []
                for g in range(2):
                    for l in range(2):
                        def km_fill(g=g, l=l):
                            nc.gpsimd.tensor_copy(out=va[g][:, :, l, 0:64],
                                                  in_=km_sb)
                        th.append(km_fill)
                for t0 in range(0, NTK, 2):
                    w = min(2, NTK - t0)
                    grp = [None]

                    def fill(t0=t0, w=w, grp=grp):
                        grp[0] = pso.tile([128, w, 256], F32, name="psv",
                                          padded_shape=[128, w, 256],
                                          tag="pso")
                        for i in range(w):
                            for d in range(DT):
                                nc.tensor.matmul(
                                    grp[0][:, i, :],
                                    lhsT=xs_sb[:, d, (t0 + i) * 128:(t0 + i + 1) * 128],
                                    rhs=wv(d),
                                    start=(d == 0), stop=(d == DT - 1))
                    th.append(fill)

                    def evac(t0=t0, w=w, grp=grp):
                        for g in range(2):
                            nc.vector.tensor_copy(
                                out=va[g][:, t0:t0 + w, :, 64:128],
                                in_=grp[0][:, :, g * 128:(g + 1) * 128])
                    th.append(evac)
                return th

            def b_scores(g, j, feed):
                cs = slice(j * qck, (j + 1) * qck)
                ets = []
                for t in range(NTK):
                    pt = pss.tile([128, 2, qck], F32,
                                  padded_shape=[128, 2, 512], tag="pss")
                    nc.tensor.matmul(pt[:, 0, :],
                                     lhsT=ka[g][0:64, t * 128:(t + 1) * 128],
                                     rhs=qa[g][0:64, cs],
                                     start=True, stop=True)
                    nc.tensor.matmul(pt[:, 1, :],
                                     lhsT=ka[g][64:128, t * 128:(t + 1) * 128],
                                     rhs=qa[g][64:128, cs],
                                     start=True, stop=True)
                    et = eb.tile([128, 2, qck], BF16, tag="et")
                    nc.scalar.activation(out=et, in_=pt, func=EXP)
                    ets.append(et)
                    feed()
                return ets

            def b_pv_nrm(g, j, ets, fine=False):
                cs = slice(j * qck, (j + 1) * qck)
                pos = []
                for l in range(2):
                    po = pso.tile([128, qck], F32, padded_shape=[128, 512],
                                  tag="pso")
                    pos.append(po)
                    for t in range(NTK):
                        nc.tensor.matmul(po, lhsT=va[g][:, t, l, :],
                                         rhs=ets[t][:, l, :],
                                         start=(t == 0), stop=(t == NTK - 1))
                for l in range(2):
                    po = pos[l]
                    if fine:
                        for u in range(QTPC):
                            us = slice(u * 128, (u + 1) * 128)
                            ucs = slice(j * qck + u * 128,
                                        j * qck + (u + 1) * 128)
                            rbu = nrm.tile([64, 128], F32, name="rbu",
                                           tag="rbu")
                            nc.vector.reciprocal_approx_fast(
                                out=rbu, in_=po[0:64, us])
                            nc.vector.tensor_mul(
                                out=otp[g][l * 64:(l + 1) * 64, ucs],
                                in0=po[64:128, us], in1=rbu)
                        continue
                    rb = nrm.tile([64, qck], F32, name="rb", tag="rb")
                    nc.vector.reciprocal_approx_fast(out=rb, in_=po[0:64, :])
                    if debug and g == 0 and j == 0:
                        dt_ = P.tile([128, 512], BF16, name=f"dbgpo{l}",
                                     tag=f"dbgpo{l}")
                        nc.scalar.copy(out=dt_[:, 0:qck], in_=po)
                        nc.sync.dma_start(out=dbg["poA" if l == 0 else "poB"],
                                          in_=dt_)
                        if l == 0:
                            dr_ = P.tile([64, 512], BF16, name="dbgrb",
                                         tag="dbgrb")
                            nc.vector.tensor_copy(out=dr_[:, 0:qck], in_=rb)
                            nc.sync.dma_start(out=dbg["rbA"], in_=dr_)
                            de_ = P.tile([128, 2, 512], BF16, name="dbge",
                                         tag="dbge")
                            nc.vector.tensor_copy(out=de_[:, :, 0:qck],
                                                  in_=ets[0])
                            nc.sync.dma_start(
                                out=dbg["e00"],
                                in_=de_.rearrange("p a b -> p (a b)"))
                    nc.vector.tensor_mul(
                        out=otp[g][l * 64:(l + 1) * 64, cs],
                        in0=po[64:128, :], in1=rb)

            def c_out(j, copy_eng, use_pss=False, copy_eng2=None,
                      dma3=False):
                for tt in range(j * QTPC, (j + 1) * QTPC):
                    yt = ytp.tile([128, d_model], BF16, tag="yt")
                    if use_pss:
                        pc2 = pss.tile([128, 2, 512], F32, name="pc2",
                                       tag="pss")
                    for c2 in range(DCH):
                        if use_pss:
                            pc = pc2[:, c2 % 2, :]
                        else:
                            pc = pso.tile([128, 512], F32, name="pc",
                                          tag="pso")
                        for b in range(2):
                            nc.tensor.matmul(
                                pc,
                                lhsT=otp[b][:, tt * 128:(tt + 1) * 128],
                                rhs=wo_sb[:, b, c2 * 512:(c2 + 1) * 512],
                                start=(b == 0), stop=(b == 1))
                        eng = copy_eng if (copy_eng2 is None or c2 % 2 == 0)                             else copy_eng2
                        eng(out=yt[:, c2 * 512:(c2 + 1) * 512], in_=pc)
                    if dma3:
                        # last chunk: avoid gpsimd so no DMA is in flight at
                        # block exit (its dge_drain then costs ~3.4us).
                        dq = nc.sync if tt % 2 == 0 else nc.scalar
                    else:
                        dq = nc.sync if tt % 2 == 0 else nc.gpsimd
                    dq.dma_start(out=y_d[tt * 128:(tt + 1) * 128, :], in_=yt)

            # ---------------- program ----------------
            def vector_copy(out, in_):
                nc.vector.tensor_copy(out=out, in_=in_)

            # Stage A pair 0: K and Q t-major across 6 held PSUM slots so
            # the projections finish right after the last xs chunk lands.
            # Evacuations on Vector; Scalar is reserved for exp.
            if NCHK <= 3 and NCHQ <= 3:
                psK2 = pss.tile([128, 2, 512], F32, name="psK2", tag="pss")
                psQ2 = pss.tile([128, 2, 512], F32, name="psQ2", tag="pss")
                psK1 = pso.tile([128, kck], F32, name="psqk",
                                padded_shape=[128, 512], tag="pso")
                psQ1 = pso.tile([128, qck], F32, name="psqk",
                                padded_shape=[128, 512], tag="pso")
                kslots = [psK2[:, 0, 0:kck], psK2[:, 1, 0:kck], psK1][:NCHK]
                qslots = [psQ2[:, 0, 0:qck], psQ2[:, 1, 0:qck], psQ1][:NCHQ]
                for t in range(DT):
                    for jk in range(NCHK):
                        nc.tensor.matmul(
                            kslots[jk], lhsT=wk(t, 0),
                            rhs=xs_sb[:, t, jk * kck:(jk + 1) * kck],
                            start=(t == 0), stop=(t == DT - 1))
                    for j in range(NCHQ):
                        nc.tensor.matmul(
                            qslots[j], lhsT=wq(t, 0),
                            rhs=xs_sb[:, t, j * qck:(j + 1) * qck],
                            start=(t == 0), stop=(t == DT - 1))
                for jk in range(NCHK):
                    vector_copy(out=ka[0][:, jk * kck:(jk + 1) * kck],
                                in_=kslots[jk])
                for j in range(NCHQ):
                    vector_copy(out=qa[0][:, j * qck:(j + 1) * qck],
                                in_=qslots[j])
            else:  # generic fallback for unusual mask statistics
                for j in range(NCHQ):
                    emit_qk(0, 0, j, vector_copy)
                for j in range(NCHK):
                    emit_qk(0, 1, j, vector_copy)

            # Emission order tuned so the exp stream (Scalar, the critical
            # engine) is fed as early and as continuously as possible while
            # V/A(pair1)/PV/C soak up the PE between score bursts.
            # Pair-interleaved chunks: per chunk j run B(g0,j), B(g1,j),
            # then C(j) while later chunks' exps still cover the PE. Only the
            # last chunk's PV+C sit past the final exp.
            none = lambda: None
            e00 = b_scores(0, 0, none)            # exps start here
            for j in range(NCHQ):                 # A.Q(pair 1)
                emit_qk(1, 0, j, vector_copy)
            for j in range(NCHK):                 # A.K(pair 1)
                emit_qk(1, 1, j, vector_copy)
            e10 = b_scores(1, 0, none)
            for th in v_thunks():                 # V projection, both pairs
                th()
            # w_out lands only now: it is not needed before stage C and must
            # not compete with xs/wqk for DMA bandwidth during the lead-in.
            nc.gpsimd.dma_start(
                out=wo_sb.rearrange("p a b -> p (a b)"), in_=wc_d[:, DT * 768:])
            b_pv_nrm(0, 0, e00)
            e01 = b_scores(0, 1, none)
            b_pv_nrm(1, 0, e10)
            e11 = b_scores(1, 1, none)
            c_out(0, nc.vector.tensor_copy)
            b_pv_nrm(0, 1, e01)
            e02 = b_scores(0, 2, none)
            b_pv_nrm(1, 1, e11)
            c_out(1, nc.vector.tensor_copy)
            e12 = b_scores(1, 2, none)
            b_pv_nrm(0, 2, e02)
            b_pv_nrm(1, 2, e12, fine=True)
            c_out(2, nc.scalar.copy, use_pss=True,
                  copy_eng2=nc.vector.tensor_copy, dma3=True)

            if debug:
                nc.sync.dma_start(out=dbg["qa0"], in_=qa[0])
                nc.sync.dma_start(out=dbg["ka0"], in_=ka[0])
                nc.sync.dma_start(
                    out=dbg["va0"],
                    in_=va[0].rearrange("p a b c -> p (a b c)"))
                nc.sync.dma_start(out=dbg["otp0"], in_=otp[0])
                nc.sync.dma_start(out=dbg["otp1"], in_=otp[1])

    nc.compile()
    return nc


_NC_CACHE = {}


def _get_nc(n=2048, d_model=1024, nq=None, nk=None):
    key = (n, d_model, nq, nk)
    if key not in _NC_CACHE:
        _NC_CACHE[key] = build_nc(n, d_model, nq=nq, nk=nk)
    return _NC_CACHE[key]


def _pick_nq(mask, n):
    """Kept-query count: smallest multiple of 384 covering max(m1)."""
    m1max = int(np.asarray(mask).astype(bool).sum(axis=1).max())
    nq = ((m1max + 383) // 384) * 384
    return min(nq, n)


def _pick_nk(mask, n):
    """Kept-key count: smallest multiple of 128 covering max(m1)."""
    m1max = int(np.asarray(mask).astype(bool).sum(axis=1).max())
    nk = ((m1max + 127) // 128) * 128
    return min(nk, n)


def make_in_maps(x, mask, w_qkv, w_out, nq=None, nk=None):
    """Host-side sharding: per-core input dict."""
    x = np.asarray(x, dtype=np.float32)
    mask = np.asarray(mask)
    w_qkv = np.asarray(w_qkv, dtype=np.float32)
    w_out = np.asarray(w_out, dtype=np.float32)
    b, n, dim = x.shape
    inner = HEADS * DH
    hl = HEADS // 4                      # 4 heads per core
    hw = hl * DH                         # 256 inner cols per core
    import ml_dtypes
    bf16 = ml_dtypes.bfloat16
    maskf = mask.astype(np.float32)
    if nq is None:
        nq = _pick_nq(mask, n)
    if nk is None:
        nk = _pick_nk(mask, n)
    nx = max(nq, nk)
    NTK = nk // 128
    DT = dim // 128
    in_maps = []
    orders = [np.argsort(-maskf[bc], kind="stable") for bc in range(b)]
    for c in range(N_CORES):
        bc, hg = divmod(c, 4)
        rs = slice(hg * hw, (hg + 1) * hw)
        wq = w_qkv[0 * inner:1 * inner, :][rs, :]
        wk = w_qkv[1 * inner:2 * inner, :][rs, :]
        wv = w_qkv[2 * inner:3 * inner, :][rs, :]
        order = orders[bc]
        m1 = int(maskf[bc].sum())
        # sorted x with masked-position rows zeroed, first nx positions
        xz = x[bc][order[:nx], :].copy()
        xz[m1:, :] = 0.0
        xp = (xz.T.reshape(DT, 128, nx).transpose(1, 0, 2)
              .reshape(128, DT * nx).astype(bf16))
        # key mask, partition-major per tile, replicated 64x: [128, NTK, 64]
        kms = maskf[bc][order[:nk]].reshape(NTK, 128).T   # [128, NTK]
        km64 = np.repeat(kms[:, :, None], 64, axis=2).reshape(128, NTK * 64)
        # wcat: per partition p, per dim tile t, q|k|v rows for dim t*128+p,
        # then the packed w_out rows
        wqk = np.stack([(wq.T * np.float32(SCALE)).reshape(DT, 128, hw),
                        wk.T.reshape(DT, 128, hw)], axis=2)   # [DT,128,2,hw]
        wflat = np.concatenate(
            [wqk.transpose(1, 0, 2, 3).reshape(128, DT * 2 * hw),
             wv.T.reshape(DT, 128, hw).transpose(1, 0, 2).reshape(128, DT * hw)],
            axis=1)
        NB = hw // 128
        wop = (w_out[:, rs].T.reshape(NB, 128, dim).transpose(1, 0, 2)
               .reshape(128, NB * dim))
        wcat = np.concatenate([wflat, wop], axis=1).astype(bf16)
        in_maps.append({
            "xs": np.ascontiguousarray(xp),
            "wcat": np.ascontiguousarray(wcat),
            "km64": np.ascontiguousarray(km64.astype(bf16)),
        })
    return in_maps


def gather(results, mask, x, w_qkv, w_out, b=2, n=2048, dim=1024, nq=None):
    """Sum the 4 head-group partials per batch and undo the query sort.

    All fully-masked queries share one output row (uniform attention over
    all keys = mean of v over all positions); that row is computed here on
    the host, so the device never handles masked queries."""
    maskf = np.asarray(mask).astype(np.float32)
    x = np.asarray(x, dtype=np.float32)
    w_qkv = np.asarray(w_qkv, dtype=np.float32)
    w_out = np.asarray(w_out, dtype=np.float32)
    inner = HEADS * DH
    wv_full = w_qkv[2 * inner:3 * inner, :]
    if nq is None:
        nq = _pick_nq(mask, n)
    y = np.zeros((b, n, dim), dtype=np.float32)
    for bc in range(b):
        yk = np.zeros((nq, dim), dtype=np.float32)
        for c in range(N_CORES):
            if c // 4 == bc:
                yk += np.asarray(results[c]["y"], dtype=np.float32)
        order = np.argsort(-maskf[bc], kind="stable")
        m1 = int(maskf[bc].sum())
        y[bc][order[:m1]] = yk[:m1]
        if m1 < n:
            vmean = (x[bc].sum(axis=0) @ wv_full.T) / np.float32(n)
            y[bc][order[m1:]] = vmean @ w_out.T
    return y


def run(x, mask, w_qkv, w_out, trace=False, trace_cores=None):
    b, n, dim = np.asarray(x).shape
    nq = _pick_nq(mask, n)
    nk = _pick_nk(mask, n)
    nc = _get_nc(n=n, d_model=dim, nq=nq, nk=nk)
    in_maps = make_in_maps(x, mask, w_qkv, w_out, nq=nq, nk=nk)
    res = run_bass_kernel_spmd(nc, in_maps, core_ids=list(range(N_CORES)),
                               trace=trace, trace_cores=trace_cores)
    return gather(res.results, mask, x, w_qkv, w_out,
                  b=b, n=n, dim=dim, nq=nq), res


def kernel(x, mask, w_qkv, w_out):
    y, _ = run(x, mask, w_qkv, w_out)
    return y
